# revision 75
# baseline (speedup 1.0000x reference)
"""Trainium2 Bass kernel for a single attention layer.

Problem: x[4,2048,512], W_q/W_k/W_v[512,512], b_q/b_k/b_v[512]
  q = x@W_q+b_q; k = x@W_k+b_k; v = x@W_v+b_v
  out = softmax(q @ k.T) @ v          (per batch)

Sharding: 8 cores = 4 batches x 2 sequence-halves (data parallel).
Each core receives its batch's full x with its query-half rolled to the
front (key order is permutation-invariant under softmax-attention), and
computes the output rows for its 1024 queries.

v8: W_v reassociation — out = (P @ x) @ W_v + b_v instead of
P @ (x W_v).  This removes the per-core V projection (which was
duplicated across the two sequence-half cores) and reaches the global
MAC roofline: 2.684e9 MACs/core = 163840 PE cycles at fp16, gap-free.
  - AT[d,q] = sum_k x[k,d] P[q,k] is produced directly by PE with
    x (natural layout, new xs input) as stationary and PT as moving
    (N=128); LdWeights is free.
  - b_v is added on the HOST after the gather; 1/rowsum (from exp
    accum_out) folds into the W-stage eviction as a per-partition
    scale, so nothing but a single evict+store rides the tail.
  - PSUM: 6-bank "sc" ring (scores/proj/warmup/po) + dedicated 2-bank
    atp ring so A(qt+1) never waits on A(qt)'s eviction.
  - Engine queues are in-order: proj bias-evicts and non-tail W-evicts
    run on DVE, exps/AT-evicts on ACT, emission ordered so no op waits
    head-of-line on a later dependency.
  - DMA: host packs [M et0 | xT 0:256 | M et1..3 | xT 256:], so piece 1
    (0.375MB) lands ~3.96us and Qproj et0 starts at warmup end; the
    full-M gate (0.75MB, ~5.05us) bounds et1 — the hard startup floor.
    wv rides SP after the xm pieces (a Pool-SWDGE wv would FIFO ahead
    of the last xm piece on the single-slot DMA engines); xs in 4
    pieces.  Scores chunk 0 straddles the repack split and uses two
    column-group accumulations (same cycles).
  - A dummy exp at build start pulls the 1.3us LoadActFuncSet (Exp
    table) into the DMA-wait window instead of delaying exp(0).
  - Tail: last W-stage splits into two column-group PSUM tiles so the
    first half's ACT eviction overlaps the second half's matmuls; last
    two stores on SP/HWDGE (fast descriptor gen).
Everything else (all-fp16 datapath, softmax reduction M = W_q W_k^T,
u = W_k b_q, warm-up matmuls bridging the PE p-state ramp to the first
real dispatch) is inherited from v6.
"""
import sys

sys.path.insert(0, "/opt/trn_rl_repo")

import numpy as np
from contextlib import ExitStack

B, S, D = 4, 2048, 512
SQ = S // 2          # queries per core
P = 128              # partitions
DT = D // P          # 4 d-tiles
NT = S // P          # 16 s-tiles
QT_N = SQ // P       # 8 q-tiles per core
KC = S // 512        # 4 key chunks of 512
N_CORES = 8

_NC_CACHE = None


def _build_nc(reps=1):
    import concourse.bacc as bacc
    import concourse.tile as tile
    from concourse import mybir

    f32 = mybir.dt.float32
    f16 = mybir.dt.float16
    AF = mybir.ActivationFunctionType
    X = mybir.AxisListType.X

    nc = bacc.Bacc(trn_type="TRN2")

    # host-packed [M[:,0:128] | x^T[:,0:256] | M[:,128:512] | x^T[:,256:]]:
    # the first 384 columns are everything Qproj's first et-group needs,
    # so the critical first DMA piece is 0.375MB and lands ~3.96us
    xm_d = nc.dram_tensor("xm", [D, D + S], f16, kind="ExternalInput")
    xs_d = nc.dram_tensor("xs", [S, D], f16, kind="ExternalInput")
    wv_d = nc.dram_tensor("wv", [D, D], f16, kind="ExternalInput")
    u_d = nc.dram_tensor("u", [D], f32, kind="ExternalInput")
    out_d = nc.dram_tensor("out", [SQ, D], f16, kind="ExternalOutput")

    with tile.TileContext(nc) as tc, ExitStack() as ctx:
        persist = ctx.enter_context(tc.tile_pool(name="persist", bufs=1))
        ppool = ctx.enter_context(tc.tile_pool(name="ppool", bufs=4))
        ptpool = ctx.enter_context(tc.tile_pool(name="ptpool", bufs=4))
        atpool = ctx.enter_context(tc.tile_pool(name="atpool", bufs=3))
        opool = ctx.enter_context(tc.tile_pool(name="opool", bufs=4))
        stat = ctx.enter_context(tc.tile_pool(name="stat", bufs=5))
        # "sc" ring (6 banks): score chunks, projections, warmup, and the
        # W-stage po outputs.  Dedicated 2-bank atp ring so atp(qt+1) never
        # waits on A(qt)'s eviction (ring alternation = one full iteration
        # of slack).  6 + 2 = all 8 PSUM banks.
        psS = ctx.enter_context(tc.tile_pool(name="psS", bufs=6, space="PSUM"))
        psM = psS
        psA = ctx.enter_context(tc.tile_pool(name="psA", bufs=2, space="PSUM"))

        for _rep in range(reps):
            # ---- persistent SBUF tensors ---------------------------------
            xm = persist.tile([P, DT, D + S], f16)
            # repacked layout: cols 0:128 = M et0, 128:384 = xT seq 0:256,
            # 384:768 = M et1..3, 768:2560 = xT seq 256:2048
            xT0 = xm[:, :, 128:384]

            def mW(et):
                return (xm[:, :, 0:P] if et == 0
                        else xm[:, :, 256 + et * P:256 + (et + 1) * P])

            def xTr(lo, hi):
                # xT columns for seq range [lo, hi), lo >= 256
                return xm[:, :, 512 + lo:512 + hi]

            XS = persist.tile([P, NT, D], f16)
            QT = persist.tile([P, DT, SQ], f16)

            # ---- PE warm-up ----------------------------------------------
            # The cost model prices each matmul off the length of PE's
            # current busy streak at dispatch; everything in the first 3us
            # of a streak runs below 2.4GHz. Dummy matmuls on a zeroed tile
            # keep PE busy from ~1.2us so the real projections (dispatching
            # when xT lands ~5.8us) are priced at full clock.
            warm = persist.tile([P, 512], f16, tag="warm")
            nc.gpsimd.memset(warm, 0.0)
            # dummy exp pulls the 1.3us LoadActFuncSet (Exp table) into the
            # idle DMA-wait window instead of delaying exp(0) at ~12.5us
            dumm = stat.tile([P, 1], f32, tag="dumm", name="dumm")
            nc.scalar.activation(out=dumm, in_=warm[:, 0:1], func=AF.Exp)
            for _ in range(7):
                wp = psS.tile([P, 512], f32, tag="sc")
                nc.tensor.matmul(wp, warm[:, 0:P], warm, start=True, stop=True)

            u_sb = persist.tile([P, DT], f32)

            # SP-queue DMAs in consumption order off the host-packed
            # tensor, then x natural (xs) for the attend stage; u rides
            # Pool SWDGE in parallel.  Piece 1 = M et0 + xT[0:256], so the
            # first Qproj group dispatches at warmup end (~4.25us, mid
            # clock) and the rest run full-clock as later pieces land.
            for lo, hi in ((0, 384), (384, 768), (768, 1024), (1024, 1536),
                           (1536, 2048), (2048, D + S)):
                nc.sync.dma_start(
                    out=xm[:, 0:DT, lo:hi],
                    in_=xm_d.ap()[:, lo:hi].rearrange("(t p) s -> p t s", p=P),
                )
            # wv rides the SP queue AFTER the xm pieces: issued from Pool
            # SWDGE its transfer would slot into the DMA FIFO ahead of the
            # last xm piece and delay S0's final key chunk by ~1.7us
            wv_t = persist.tile([P, DT, D], f16, tag="w_wv")
            nc.sync.dma_start(
                out=wv_t, in_=wv_d.ap().rearrange("(t p) e -> p t e", p=P))
            def load_xs(lo, hi):
                nc.sync.dma_start(
                    out=XS[:, lo:hi, :],
                    in_=xs_d.ap()[lo * P:hi * P, :].rearrange(
                        "(t p) e -> p t e", p=P),
                )

            load_xs(0, 4)
            load_xs(4, 8)
            # xs pieces 3/4 are emitted after emit_exp(0) below, so PT0's
            # latency-critical transposes slot into the DMA FIFO ahead of
            # them instead of queueing behind the whole input stream
            nc.gpsimd.dma_start(out=u_sb, in_=u_d.ap().rearrange("(t p) -> p t", p=P))

            # ---- Q' projection, chunk-pipelined --------------------------
            def proj_chunk(lo, hi):
                xt = xT0 if hi <= 256 else xTr(lo, hi)
                for et in range(DT):
                    pp = psM.tile([P, 512], f32, tag="sc")
                    for dt in range(DT):
                        nc.tensor.matmul(
                            pp[:, 0:hi - lo],
                            mW(et)[:, dt, :],
                            xt[:, dt, 0:hi - lo] if hi <= 256
                            else xt[:, dt, :],
                            start=(dt == 0), stop=(dt == DT - 1),
                        )
                    # bias-evict on DVE (idle early): keeps ACT free for
                    # exp(0) and recycles the "sc" PSUM ring promptly
                    nc.vector.tensor_scalar_add(
                        out=QT[:, et, lo:hi], in0=pp[:, 0:hi - lo],
                        scalar1=u_sb[:, et:et + 1],
                    )

            # ---- attention per q-tile ------------------------------------
            state = {}

            def emit_scores(qt):
                sc = []
                mx_part = stat.tile([P, KC], f32, tag="mx")
                for kcc in range(KC):
                    ss = psS.tile([P, 512], f32, tag="sc")
                    if kcc == 0:
                        # keys 0:512 straddle the repack split at seq 256:
                        # two column-group accumulations over et each
                        for cs, xt in ((slice(0, 256), xT0),
                                       (slice(256, 512), xTr(256, 512))):
                            for et in range(DT):
                                nc.tensor.matmul(
                                    ss[:, cs],
                                    QT[:, et, qt * P:(qt + 1) * P],
                                    xt[:, et, :],
                                    start=(et == 0), stop=(et == DT - 1),
                                )
                    else:
                        for et in range(DT):
                            nc.tensor.matmul(
                                ss,
                                QT[:, et, qt * P:(qt + 1) * P],
                                xTr(kcc * 512, (kcc + 1) * 512)[:, et, :],
                                start=(et == 0), stop=(et == DT - 1),
                            )
                    nc.vector.reduce_max(out=mx_part[:, kcc:kcc + 1], in_=ss, axis=X)
                    sc.append(ss)
                negmax = stat.tile([P, 1], f32, tag="negmax")
                nc.vector.reduce_max(out=negmax, in_=mx_part, axis=X, negate=True)
                state[qt] = (sc, negmax)

            def emit_exp(qt):
                # exp chunks with accum_out rowsums; each PT half transposes
                # right after its two exp chunks.  1/rowsum is folded into
                # the W-stage eviction (per-partition there), keeping the
                # exp->PT chain short.
                sc, negmax = state.pop(qt)
                p_sb = ppool.tile([P, S], f16, tag="P")
                PT = ptpool.tile([P, NT, P], f16, tag="PT")
                rs_part = stat.tile([P, KC], f32, tag="rs", name="rs_part")
                for kcc in range(KC):
                    nc.scalar.activation(
                        out=p_sb[:, kcc * 512:(kcc + 1) * 512], in_=sc[kcc],
                        func=AF.Exp, bias=negmax, scale=1.0,
                        accum_out=rs_part[:, kcc:kcc + 1],
                    )
                    if kcc % 2 == 1:
                        h = kcc // 2
                        nc.sync.dma_start_transpose(
                            out=PT[:, h * 8:(h + 1) * 8, :],
                            in_=p_sb[:, h * 1024:(h + 1) * 1024],
                        )
                rowsum = stat.tile([P, 1], f32, tag="rowsum")
                nc.vector.reduce_sum(out=rowsum, in_=rs_part, axis=X)
                recip = stat.tile([P, 1], f32, tag="recip")
                nc.vector.reciprocal(recip, rowsum)
                state[qt] = (PT, recip)

            def emit_at(qt):
                # AT[d-local, dt, q] = sum_k x[k, dt*128+d] P[q, k]
                PT, recip = state.pop(qt)
                atp = psA.tile([P, DT, P], f32, tag="atp")
                for dt in range(DT):
                    for kt in range(NT):
                        nc.tensor.matmul(
                            atp[:, dt, :],
                            XS[:, kt, dt * P:(dt + 1) * P],
                            PT[:, kt, :],
                            start=(kt == 0), stop=(kt == NT - 1),
                        )
                at_sb = atpool.tile([P, DT, P], f16, tag="at")
                nc.scalar.copy(out=at_sb, in_=atp)
                state[qt] = (at_sb, recip)

            def emit_wv(qt, tail=False, store_sp=False):
                # b_v is added on the host; 1/rowsum is a per-partition
                # scale at eviction (same op cost as a plain cast).
                at_sb, recip = state.pop(qt)
                o_sb = opool.tile([P, D], f16, tag="o")
                if tail:
                    # two column-group accumulations in separate PSUM tiles
                    # (separate memrefs — no false WAR with the eviction):
                    # the first half's ACT eviction overlaps the second
                    # half's matmuls, so the final store dispatches ~450ns
                    # after the last matmul
                    for h in range(2):
                        cs = slice(h * 256, (h + 1) * 256)
                        ph = psS.tile([P, 256], f32, tag="sc", name="po")
                        for dt in range(DT):
                            nc.tensor.matmul(
                                ph, at_sb[:, dt, :], wv_t[:, dt, cs],
                                start=(dt == 0), stop=(dt == DT - 1),
                            )
                        nc.scalar.mul(out=o_sb[:, cs], in_=ph, mul=recip)
                else:
                    po = psS.tile([P, D], f32, tag="sc", name="po")
                    for dt in range(DT):
                        nc.tensor.matmul(
                            po, at_sb[:, dt, :], wv_t[:, dt, :],
                            start=(dt == 0), stop=(dt == DT - 1),
                        )
                    # non-tail evicts on DVE so they never queue on ACT
                    # behind a later AT eviction (keeps their stores clear
                    # of the final store's DMA window)
                    nc.vector.tensor_scalar_mul(out=o_sb, in0=po, scalar1=recip)
                if tail or store_sp:
                    # SP/HWDGE store: fast descriptor gen for the last tiles
                    nc.sync.dma_start(
                        out=out_d.ap()[qt * P:(qt + 1) * P, :], in_=o_sb,
                    )
                else:
                    # store via Pool SWDGE: keeps the HWDGE lane ring (shared
                    # by latency-critical PT transposes) decoupled
                    nc.gpsimd.dma_start(
                        out=out_d.ap()[qt * P:(qt + 1) * P, :], in_=o_sb,
                    )

            # Prologue runs 4 score tiles ahead so each A(qt) sits well
            # behind its exp/normalize/PT chain; W(qt) trails A(qt) by one
            # PE group so the AT eviction is off the critical path.
            # S0 needs only Q-tile 0 (proj chunk A) and full keys; running
            # it before proj chunk C starts the exp(0)->PT0 chain ~3.4us
            # earlier so A0 never stalls.  exp(0) is emitted before chunk
            # C so its ACT exps aren't queued behind chunk C's evictions;
            # chunk C's QT columns are only needed from S4 (~25us).
            proj_chunk(0, 256)
            proj_chunk(256, 512)
            emit_scores(0)
            emit_exp(0)
            load_xs(8, 12)
            load_xs(12, 16)
            proj_chunk(512, 1024)
            for qt in range(1, 4):
                emit_scores(qt)
                emit_exp(qt)
            emit_at(0)
            # Emission order within an iteration: scores, exp (prompt ACT
            # exps must precede the A-end-gated AT eviction in ACT's
            # in-order queue), then attend and wv-stage.
            for qt in range(1, 5):
                emit_scores(qt + 3)
                emit_exp(qt + 3)
                emit_at(qt)
                emit_wv(qt - 1)
            # A5 A6 W4 A7 W5 W6 W7: two W groups between A7 and W7 cover
            # the AT(7) eviction latency so W7 dispatches without a stall
            emit_at(5)
            emit_at(6)
            emit_wv(4)
            emit_at(7)
            emit_wv(5)
            emit_wv(6, store_sp=True)
            emit_wv(7, tail=True)

    nc.finalize()
    return nc


def _shard_inputs(x, W_q, W_k, W_v, b_q, b_k, b_v):
    xb = x.astype(np.float16)
    # softmax-invariant reduction: scores ~ (x M + u) x^T
    m = (W_q.astype(np.float64) @ W_k.astype(np.float64).T).astype(np.float16)
    u = (W_k.astype(np.float64) @ b_q.astype(np.float64)).astype(np.float32)
    wv = W_v.astype(np.float16)
    in_maps = []
    for c in range(N_CORES):
        b, h = divmod(c, 2)
        xc = xb[b]
        xk = xc if h == 0 else np.concatenate([xc[SQ:], xc[:SQ]], axis=0)
        xkT = xk.T
        in_maps.append({
            # [M et0 | xT seq 0:256 | M et1..3 | xT seq 256:2048]
            "xm": np.ascontiguousarray(np.concatenate(
                [m[:, 0:128], xkT[:, 0:256], m[:, 128:512], xkT[:, 256:]],
                axis=1)),
            "xs": np.ascontiguousarray(xk),
            "wv": wv, "u": u,
        })
    return in_maps


def kernel(x, W_q, W_k, W_v, b_q, b_k, b_v):
    from concourse.bass_utils import run_bass_kernel_spmd

    global _NC_CACHE
    if _NC_CACHE is None:
        _NC_CACHE = _build_nc()
    nc = _NC_CACHE

    args = [np.ascontiguousarray(np.asarray(a, dtype=np.float32))
            for a in (x, W_q, W_k, W_v, b_q, b_k, b_v)]
    in_maps = _shard_inputs(*args)

    res = run_bass_kernel_spmd(nc, in_maps, core_ids=list(range(N_CORES))).results

    out = np.empty((B, S, D), dtype=np.float32)
    for c in range(N_CORES):
        b, h = divmod(c, 2)
        # b_v is folded in on the host: out = (P/rs) @ x @ W_v + b_v
        out[b, h * SQ:(h + 1) * SQ] = res[c]["out"].astype(np.float32) + args[6]
    return out
